# revision 27
# baseline (speedup 1.0000x reference)
"""Trainium2 Bass kernel for nn_AttentiveTransformer (TabNet attentive transformer).

Computes, for full inputs (N=16384, NA=256, F=2048):
    x  = a @ W.T + b
    xn = batchnorm(x)  (training mode, batch stats over all N rows)
    m  = sparsemax_ascending_variant(xn * ps)
    new_ps = ps * (1.5 - m)

Key identities:
 * The reference "sparsemax" sorts ascending; its k_z condition always fires
   at the top index, so k_z = D-1 exactly and tau = (sum(z)+1)/(D-1),
   m = relu(z - tau). No sort needed.
 * BN stats from Gram partials: S1[f] = colsum(a).W_f, S2[f] = diag(W G W^T).
   var = S2/N - (S1/N)^2; the affine normalization folds into the matmul:
   W' = W*s, bias t = bn_b - (S1/N)*s (b cancels).
 * Everything runs in bf16 (operands + outputs, fp32 PSUM accumulation);
   measured rel err ~8e-3 vs the 2e-2 gate.
 * No collective: every core computes the FULL Gram locally from the whole
   a matrix (a is only 8.4MB in bf16). The 16KB AllReduce this replaces
   costs ~75us of fixed barrier+trigger+mesh latency on this platform.

Sharding: data-parallel over rows for the main pass, 2048 rows/core.
"""

import os
import sys
import numpy as np

for _p in ("/opt/trn_rl_repo",):
    if _p not in sys.path:
        sys.path.insert(0, _p)

KVAR = os.environ.get("KVAR", "")             # debug variant flags

N, NA, F = 16384, 256, 2048
NCORES = 8
NSH = N // NCORES            # 2048 rows per core
P = 128                      # partitions
RT = NSH // P                # 16 row-tiles per core
FCW = 512                    # feature chunk width (psum bank / max moving free)
FC = F // FCW                # 4 feature chunks
FP = F // P                  # 16 (cols of the [128,16] stats layout)
NAUG = NA + 1                # 257: a with ones column (colsum rides the Gram)
GAMMA = 1.5
BN_EPS = 1e-5
INV_D1 = 1.0 / (F - 1.0)     # 1/2047
ACH = N // P                 # 128 row-chunks of the full a matrix

_CACHE = {}


def _build_bass():
    import concourse.mybir as mybir
    import concourse.tile as tile
    from concourse import bacc
    from concourse.bass import ts

    fp32 = mybir.dt.float32
    bf16 = mybir.dt.bfloat16
    fp16 = mybir.dt.float16
    Alu = mybir.AluOpType
    Act = mybir.ActivationFunctionType

    nc = bacc.Bacc(
        "TRN2",
        target_bir_lowering=False,
        debug=False,
        enable_asserts=False,
        num_devices=NCORES,
    )

    # I/O (per core). af_blk = FULL a in bf16 with a ones column appended
    # (replicated to every core; feeds the local full-Gram), pre-blocked on
    # the host as [group, partition, rows-per-group, col] so each DMA lands
    # 4KB-contiguous per partition (row-major a would give 514B descriptors).
    # The Gram doesn't care which rows share a partition chunk, only that
    # every row is accumulated exactly once. ahT = this core's row-shard,
    # transposed (main matmul lhsT).
    # fp8 + DoubleRow: each matmul contracts a 256-row chunk (2 stacked
    # 128-row k-subtiles) at 0.5 cyc/col. row = (g*GJ2 + j)*256 + sub*128 + p
    GRP, GJ2 = 16, N // (16 * 256)   # 16 groups x 4 chunks-of-256
    NAUGP = 272                      # 257 padded to 16-elem alignment for DR
    fp8 = mybir.dt.float8e4
    af_blk = nc.dram_tensor("af_blk", [GRP, P, GJ2, 2, NAUGP], fp8,
                            kind="ExternalInput").ap()
    ahT = nc.dram_tensor("ahT", [NA, NSH], bf16, kind="ExternalInput").ap()
    wT = nc.dram_tensor("wT", [NA, F], bf16, kind="ExternalInput").ap()
    ps_in = nc.dram_tensor("ps_in", [NSH, F], bf16, kind="ExternalInput").ap()
    bnw16 = nc.dram_tensor("bnw16", [P, FP], fp32, kind="ExternalInput").ap()
    bnb16 = nc.dram_tensor("bnb16", [P, FP], fp32, kind="ExternalInput").ap()
    m_out = nc.dram_tensor("m_out", [NSH, F], fp16, kind="ExternalOutput").ap()
    nps_out = nc.dram_tensor("nps_out", [NSH, F], bf16, kind="ExternalOutput").ap()

    ps_t = ps_in.rearrange("(t p) f -> t p f", p=P)
    m_t = m_out.rearrange("(t p) f -> t p f", p=P)
    nps_t = nps_out.rearrange("(t p) f -> t p f", p=P)

    with tile.TileContext(nc) as tc:
        with tc.tile_pool(name="res", bufs=1) as res:
          if True:
            pro = tc.alloc_tile_pool(name="pro", bufs=1)

            # ---------------- constants ----------------
            ones_colb = pro.tile([P, 1], bf16)
            nc.vector.memset(ones_colb, 1.0)
            ones_rowb = pro.tile([1, P], bf16)
            nc.vector.memset(ones_rowb, 1.0)
            # preload the ACT table set (Sqrt + fillers) off the critical path
            warm = pro.tile([1, 1], fp32)
            nc.vector.memset(warm, 1.0)
            nc.scalar.activation(warm, warm, Act.Sqrt)

            # PE p-state warm-up: a few throwaway matmuls so the Gram
            # starts at full clock instead of the cold 0.65GHz p-state
            wsc0 = pro.tile([P, FCW], bf16)
            nc.vector.memset(wsc0, 0.0)
            with tc.tile_pool(name="wup", bufs=1, space="PSUM") as wup:
                wp = wup.tile([P, FCW], fp32)
                for _ in range(8):
                    nc.tensor.matmul(wp, wsc0[:, 0:P], wsc0, start=True, stop=True)

            bnw_c = pro.tile([P, FP], fp32)
            nc.gpsimd.dma_start(bnw_c, bnw16)
            bnb_c = pro.tile([P, FP], fp32)
            nc.gpsimd.dma_start(bnb_c, bnb16)

            # ---------------- phase 1: FULL Gram, local (bf16) ----------
            # G_aug = a_aug^T a_aug over all N rows; column NA of a_aug is
            # ones, so column NA of G_aug is colsum(a).
            g0 = pro.tile([P, NA], bf16)
            g1 = pro.tile([P, NA], bf16)
            sc0 = pro.tile([P, 1], bf16)
            sc1 = pro.tile([P, 1], bf16)
            with tc.tile_pool(name="pro1", bufs=1, space="PSUM") as pp1, \
                 tc.tile_pool(name="abig", bufs=4) as abigp:
                DR = mybir.MatmulPerfMode.DoubleRow
                pg0 = pp1.tile([P, NAUGP], fp32)
                pg1 = pp1.tile([P, NAUGP], fp32)
                for g in range(GRP):
                    hch = abigp.tile([P, GJ2, 2, NAUGP], fp8, name="hch")
                    nc.sync.dma_start(hch, af_blk[g])
                    for j in range(GJ2):
                        first = g == 0 and j == 0
                        last = g == GRP - 1 and j == GJ2 - 1
                        a_t = hch[:, j, :, :]
                        nc.tensor.matmul(pg0, a_t[:, :, ts(0, P)], a_t,
                                         start=first, stop=last, perf_mode=DR)
                        nc.tensor.matmul(pg1, a_t[:, :, ts(1, P)], a_t,
                                         start=first, stop=last, perf_mode=DR)
                # W^T resident halves (phase 2 rhs + scale-fold source);
                # emitted after the af stream so they don't delay the Gram
                w0 = res.tile([P, F], bf16)
                nc.sync.dma_start(w0, wT[0:P, :])
                w1 = res.tile([P, F], bf16)
                nc.sync.dma_start(w1, wT[P:NA, :])
                # evict G + colsum as bf16
                for pg, gh, sch in ((pg0, g0, sc0), (pg1, g1, sc1)):
                    nc.vector.tensor_copy(gh, pg[:, 0:NA])
                    nc.scalar.copy(sch, pg[:, NA:NAUG])

            # ---------------- resident load of main-matmul lhsT ------------
            ah0 = res.tile([P, NSH], bf16)
            nc.sync.dma_start(ah0, ahT[0:P, :])
            ah1 = res.tile([P, NSH], bf16)
            nc.sync.dma_start(ah1, ahT[P:NA, :])

            # ---------------- phase 2: S1/S2 via H = G W^T ----------------
            st1r = pro.tile([1, F], fp32)     # S1 as a row (partition 0)
            st2r = pro.tile([1, F], fp32)     # S2 as a row (partition 0)
            with tc.tile_pool(name="pro2", bufs=1, space="PSUM") as pp2, \
                 tc.tile_pool(name="qtmp", bufs=2) as qtmp:
                for fc in range(FC):
                    fsl = ts(fc, FCW)
                    ph0 = pp2.tile([P, FCW], fp32, name="ph0", tag="ph0", bufs=2)
                    nc.tensor.matmul(ph0, g0[:, 0:P], w0[:, fsl], start=True, stop=False)
                    nc.tensor.matmul(ph0, g1[:, 0:P], w1[:, fsl], start=False, stop=True)
                    ph1 = pp2.tile([P, FCW], fp32, name="ph1", tag="ph1", bufs=2)
                    nc.tensor.matmul(ph1, g0[:, P:NA], w0[:, fsl], start=True, stop=False)
                    nc.tensor.matmul(ph1, g1[:, P:NA], w1[:, fsl], start=False, stop=True)
                    qf0 = qtmp.tile([P, FCW], fp32, name="qf0")
                    nc.vector.tensor_tensor(qf0, ph0, w0[:, fsl], Alu.mult)
                    qf1 = qtmp.tile([P, FCW], fp32, name="qf1")
                    nc.vector.tensor_tensor(qf1, ph1, w1[:, fsl], Alu.mult)
                    q0 = qtmp.tile([P, FCW], bf16, name="q0")
                    nc.scalar.copy(q0, qf0)
                    q1 = qtmp.tile([P, FCW], bf16, name="q1")
                    nc.scalar.copy(q1, qf1)
                    ps2 = pp2.tile([1, FCW], fp32, name="ps2", tag="ps2", bufs=2)
                    nc.tensor.matmul(ps2, ones_colb, q0, start=True, stop=False)
                    nc.tensor.matmul(ps2, ones_colb, q1, start=False, stop=True)
                    ps1 = pp2.tile([1, FCW], fp32, name="ps1", tag="ps1", bufs=2)
                    nc.tensor.matmul(ps1, sc0, w0[:, fsl], start=True, stop=False)
                    nc.tensor.matmul(ps1, sc1, w1[:, fsl], start=False, stop=True)
                    nc.scalar.copy(st1r[0:1, fsl], ps1)
                    nc.scalar.copy(st2r[0:1, fsl], ps2)

            # Redistribute the [1, F] rows into the [128, 16] stats layout
            # via SBUF->SBUF DMA (partition-scatter)
            st1c = pro.tile([P, FP], fp32)
            nc.gpsimd.dma_start(st1c, st1r)
            st2c = pro.tile([P, FP], fp32)
            nc.gpsimd.dma_start(st2c, st2r)

            # ---------------- phase 4: stats math in [128,16] layout --------
            sh_row = pro.tile([1, F], bf16)
            ttl = res.tile([1, F], bf16)        # folded bias row t
            ones1 = res.tile([1, P], bf16)
            nc.vector.memset(ones1, 1.0)
            with tc.tile_pool(name="smath", bufs=1) as sm:
                sq = sm.tile([P, FP], fp32)
                nc.vector.tensor_tensor(sq, st1c, st1c, Alu.mult)
                # vv = S2 - S1^2/N + N*eps  (= N*(var+eps))
                vv = sm.tile([P, FP], fp32)
                nc.vector.scalar_tensor_tensor(vv, sq, -1.0 / N, st2c, Alu.mult, Alu.add)
                nc.vector.tensor_scalar_add(vv, vv, float(N * BN_EPS))
                rr = sm.tile([P, FP], fp32)
                nc.scalar.activation(rr, vv, Act.Sqrt)
                y = sm.tile([P, FP], fp32)
                nc.vector.reciprocal(y, rr)
                # Newton iteration for 1/sqrt(vv) (ScalarE Sqrt is low-precision)
                for _ in range(1):
                    yy = sm.tile([P, FP], fp32, name="yy", tag="yy", bufs=2)
                    nc.vector.tensor_tensor(yy, y, y, Alu.mult)
                    vyy = sm.tile([P, FP], fp32, name="vyy", tag="vyy", bufs=2)
                    nc.vector.tensor_tensor(vyy, vv, yy, Alu.mult)
                    w = sm.tile([P, FP], fp32, name="w", tag="w", bufs=2)
                    nc.vector.tensor_scalar(w, vyy, -0.5, 1.5, Alu.mult, Alu.add)
                    y2 = sm.tile([P, FP], fp32, name="y2", tag="y2", bufs=2)
                    nc.vector.tensor_tensor(y2, y, w, Alu.mult)
                    y = y2
                # s = sqrt(N) * y * bn_w; folded bias t = bn_b - (S1/N)*s
                s_c = sm.tile([P, FP], fp32)
                nc.vector.scalar_tensor_tensor(s_c, y, float(np.sqrt(N)), bnw_c, Alu.mult, Alu.mult)
                tm = sm.tile([P, FP], fp32)
                nc.vector.scalar_tensor_tensor(tm, st1c, -1.0 / N, s_c, Alu.mult, Alu.mult)
                t_c = sm.tile([P, FP], fp32)
                nc.vector.tensor_tensor(t_c, tm, bnb_c, Alu.add)
                sh_c = sm.tile([P, FP], bf16)
                nc.vector.tensor_copy(sh_c, s_c)
                th_c = sm.tile([P, FP], bf16)
                nc.vector.tensor_copy(th_c, t_c)
                nc.gpsimd.dma_start(sh_row, sh_c)
                nc.gpsimd.dma_start(ttl, th_c)

            # ---------------- phase 5: fold scale into W^T ----------------
            w0h = res.tile([P, F], bf16)
            w1h = res.tile([P, F], bf16)
            with tc.tile_pool(name="pro3", bufs=2, space="PSUM") as pp3, \
                 tc.tile_pool(name="wsc", bufs=2) as wsc:
                for fc in range(FC):
                    fsl = ts(fc, FCW)
                    pb = pp3.tile([P, FCW], fp32, name="pb")
                    nc.tensor.matmul(pb, ones_rowb, sh_row[:, fsl], start=True, stop=True)
                    nc.vector.tensor_tensor(w0h[:, fsl], w0[:, fsl], pb, Alu.mult)
                    nc.vector.tensor_tensor(w1h[:, fsl], w1[:, fsl], pb, Alu.mult)

            pro.release()

            # ---------------- main loop over 16 row-tiles ----------------
            with tc.tile_pool(name="mx", bufs=8, space="PSUM") as mxp, \
                 tc.tile_pool(name="psb", bufs=8) as psb, \
                 tc.tile_pool(name="zb", bufs=4) as zb, \
                 tc.tile_pool(name="mb", bufs=3) as mb, \
                 tc.tile_pool(name="qb", bufs=3) as qb, \
                 tc.tile_pool(name="nb", bufs=3) as nb, \
                 tc.tile_pool(name="rsb", bufs=4) as rsb:
                for rt in range(RT):
                    rsl = ts(rt, P)
                    pst = psb.tile([P, F], bf16, name="pst")
                    nc.sync.dma_start(pst, ps_t[rt])
                    # fp16 zt: 2^-11 rounding is below the bf16 noise floor but
                    # unlocks DVE 2x for the downstream all-16-bit ops
                    zt = zb.tile([P, F], fp16, name="zt")
                    # pass-type-major: each lhsT is loaded once per row-tile and
                    # streams all 4 feature chunks (LDWEIGHTS dedupe-friendly)
                    px = mxp.tile([P, F], fp32, name="px", tag="px", bufs=2)
                    ptypes = [(ah0[:, rsl], w0h), (ah1[:, rsl], w1h),
                              (ones1, ttl)]
                    for pi, (lhsT, rhs) in enumerate(ptypes):
                        for fc in range(FC):
                            nc.tensor.matmul(px[:, ts(fc, FCW)], lhsT, rhs[:, ts(fc, FCW)],
                                             start=(pi == 0), stop=(pi == len(ptypes) - 1))
                    # z' = -xn * ps over the whole row-tile; rs = rowsum(z')
                    rs = rsb.tile([P, 1], fp32, name="rs")
                    nc.vector.scalar_tensor_tensor(
                        zt, px, -1.0, pst, Alu.mult, Alu.mult, accum_out=rs,
                    )
                    # rs = -sum(z); tau = (sum(z)+1)/2047 = (1-rs)/2047
                    ntau = rsb.tile([P, 1], fp32, name="ntau")      # -tau
                    nc.vector.tensor_scalar(ntau, rs, INV_D1, -INV_D1, Alu.mult, Alu.add)
                    # m = relu(z - tau) = relu(-z' + ntau); fp16 out (and
                    # fp16 m_out) so ut below runs in the DVE 4x mode
                    mt = mb.tile([P, F], fp16, name="mt")
                    nc.scalar.activation(mt, zt, Act.Relu, bias=ntau, scale=-1.0)
                    nc.scalar.dma_start(m_t[rt], mt)
                    # GAMMA - m  (exact: m = relu(z-tau) implies m >= 0)
                    ut = qb.tile([P, F], bf16, name="ut")
                    nc.vector.tensor_scalar(ut, mt, -1.0, GAMMA, Alu.mult, Alu.add)
                    nt = nb.tile([P, F], bf16, name="nt")
                    if rt % 5 == 2:
                        # DVE 2x mode (~1.2us); most tiles go to Pool so the
                        # saturated DVE keeps pace with the PE
                        nc.vector.tensor_tensor(nt, ut, pst, Alu.mult)
                    else:
                        # Pool's tensor_tensor-mult is its one tuned op (~4us)
                        nc.gpsimd.tensor_tensor(nt, ut, pst, Alu.mult)
                    nc.scalar.dma_start(nps_t[rt], nt)

    nc.compile()
    return nc


def _get_nc():
    if "nc" not in _CACHE:
        _CACHE["nc"] = _build_bass()
    return _CACHE["nc"]


def _make_in_maps(a, ps, W, b, bn_w, bn_b):
    import ml_dtypes
    bf = ml_dtypes.bfloat16
    a = np.ascontiguousarray(a, dtype=np.float32)
    ah = a.astype(bf)
    f8 = ml_dtypes.float8_e4m3
    NAUGP = 272
    af_aug = np.concatenate(
        [a.astype(f8), np.ones((N, 1), f8),
         np.zeros((N, NAUGP - NAUG), f8)], axis=1)
    # blocked fp8 layout for DoubleRow: row = (g*GJ2 + j)*256 + sub*128 + p
    GRP = 16
    GJ2 = N // (GRP * 256)
    af_blk = np.ascontiguousarray(
        af_aug.reshape(GRP, GJ2, 2, P, NAUGP).transpose(0, 3, 1, 2, 4))
    wT_np = np.ascontiguousarray(W.astype(np.float32).T.astype(bf))
    ps16 = np.ascontiguousarray(ps, dtype=np.float32).astype(bf)
    bnw16 = np.ascontiguousarray(bn_w.astype(np.float32).reshape(P, FP))
    bnb16 = np.ascontiguousarray(bn_b.astype(np.float32).reshape(P, FP))
    in_maps = []
    for c in range(NCORES):
        rows = slice(c * NSH, (c + 1) * NSH)
        in_maps.append({
            "af_blk": af_blk,
            "ahT": np.ascontiguousarray(ah[rows].T),
            "wT": wT_np,
            "ps_in": np.ascontiguousarray(ps16[rows]),
            "bnw16": bnw16,
            "bnb16": bnb16,
        })
    return in_maps


def run(a, ps, W, b, bn_w, bn_b, trace=False, **kw):
    """Run the kernel on the 8 NeuronCores; returns ((m, new_ps), BassKernelResults)."""
    from concourse import bass_utils

    nc = _get_nc()
    in_maps = _make_in_maps(a, ps, W, b, bn_w, bn_b)
    res = bass_utils.run_bass_kernel_spmd(
        nc, in_maps, core_ids=list(range(NCORES)), trace=trace, **kw,
    )
    m = np.concatenate([np.asarray(r["m_out"]) for r in res.results],
                       axis=0).astype(np.float32)
    nps = np.concatenate([np.asarray(r["nps_out"]) for r in res.results],
                         axis=0).astype(np.float32)
    return (m, nps), res


def kernel(a, ps, W, b, bn_w, bn_b):
    (m, nps), _ = run(a, ps, W, b, bn_w, bn_b, trace=False)
    return m, nps


if __name__ == "__main__":
    rng = np.random.default_rng(0)
    a = rng.standard_normal((N, NA), dtype=np.float32)
    ps = rng.random((N, F), dtype=np.float32)
    lim = 1.0 / np.sqrt(NA)
    W = rng.uniform(-lim, lim, (F, NA)).astype(np.float32)
    b = rng.uniform(-lim, lim, (F,)).astype(np.float32)
    bn_w = np.ones((F, ), np.float32)
    bn_b = np.zeros((F, ), np.float32)
    (m, nps), res = run(a, ps, W, b, bn_w, bn_b)
    print("m", m.shape, m.dtype, "nps", nps.shape)
    print("exec_time_ns:", res.exec_time_ns)


# revision 29
# speedup vs baseline: 1.3047x; 1.3047x over previous
"""Trainium2 Bass kernel for nn_AttentiveTransformer (TabNet attentive transformer).

Computes, for full inputs (N=16384, NA=256, F=2048):
    x  = a @ W.T + b
    xn = batchnorm(x)  (training mode, batch stats over all N rows)
    m  = sparsemax_ascending_variant(xn * ps)
    new_ps = ps * (1.5 - m)

Key identities:
 * The reference "sparsemax" sorts ascending; its k_z condition always fires
   at the top index, so k_z = D-1 exactly and tau = (sum(z)+1)/(D-1),
   m = relu(z - tau). No sort needed.
 * BN stats from Gram partials: S1[f] = colsum(a).W_f, S2[f] = diag(W G W^T).
   var = S2/N - (S1/N)^2; the affine normalization folds into the matmul:
   W' = W*s, bias t = bn_b - (S1/N)*s (b cancels).
 * Everything runs in bf16 (operands + outputs, fp32 PSUM accumulation);
   measured rel err ~8e-3 vs the 2e-2 gate.
 * No collective: every core computes the FULL Gram locally from the whole
   a matrix (a is only 8.4MB in bf16). The 16KB AllReduce this replaces
   costs ~75us of fixed barrier+trigger+mesh latency on this platform.

Sharding: data-parallel over rows for the main pass, 2048 rows/core.
"""

import os
import sys
import numpy as np

for _p in ("/opt/trn_rl_repo",):
    if _p not in sys.path:
        sys.path.insert(0, _p)

KVAR = os.environ.get("KVAR", "")             # debug variant flags

N, NA, F = 16384, 256, 2048
NCORES = 8
NSH = N // NCORES            # 2048 rows per core
P = 128                      # partitions
RT = NSH // P                # 16 row-tiles per core
FCW = 512                    # feature chunk width (psum bank / max moving free)
FC = F // FCW                # 4 feature chunks
FP = F // P                  # 16 (cols of the [128,16] stats layout)
NAUG = NA + 1                # 257: a with ones column (colsum rides the Gram)
GAMMA = 1.5
BN_EPS = 1e-5
INV_D1 = 1.0 / (F - 1.0)     # 1/2047
ACH = N // P                 # 128 row-chunks of the full a matrix

_CACHE = {}


def _build_bass():
    import concourse.mybir as mybir
    import concourse.tile as tile
    from concourse import bacc
    from concourse.bass import ts

    fp32 = mybir.dt.float32
    bf16 = mybir.dt.bfloat16
    fp16 = mybir.dt.float16
    Alu = mybir.AluOpType
    Act = mybir.ActivationFunctionType

    nc = bacc.Bacc(
        "TRN2",
        target_bir_lowering=False,
        debug=False,
        enable_asserts=False,
        num_devices=NCORES,
    )

    # I/O (per core). af_blk = FULL a in bf16 with a ones column appended
    # (replicated to every core; feeds the local full-Gram), pre-blocked on
    # the host as [group, partition, rows-per-group, col] so each DMA lands
    # 4KB-contiguous per partition (row-major a would give 514B descriptors).
    # The Gram doesn't care which rows share a partition chunk, only that
    # every row is accumulated exactly once. ahT = this core's row-shard,
    # transposed (main matmul lhsT).
    # fp8 + DoubleRow: each matmul contracts a 256-row chunk (2 stacked
    # 128-row k-subtiles) at 0.5 cyc/col. row = (g*GJ2 + j)*256 + sub*128 + p
    GRP, GJ2 = 16, N // (16 * 256)   # 16 groups x 4 chunks-of-256
    NAUGP = 272                      # 257 padded to 16-elem alignment for DR
    fp8 = mybir.dt.float8e4
    af_blk = nc.dram_tensor("af_blk", [GRP, P, GJ2, 2, NAUGP], fp8,
                            kind="ExternalInput").ap()
    ahT = nc.dram_tensor("ahT", [NA, NSH], bf16, kind="ExternalInput").ap()
    wT = nc.dram_tensor("wT", [NA, F], bf16, kind="ExternalInput").ap()
    ps_in = nc.dram_tensor("ps_in", [NSH, F], bf16, kind="ExternalInput").ap()
    bnw16 = nc.dram_tensor("bnw16", [P, FP], fp32, kind="ExternalInput").ap()
    bnb16 = nc.dram_tensor("bnb16", [P, FP], fp32, kind="ExternalInput").ap()
    m_out = nc.dram_tensor("m_out", [NSH, F], fp16, kind="ExternalOutput").ap()
    nps_out = nc.dram_tensor("nps_out", [NSH, F], bf16, kind="ExternalOutput").ap()

    ps_t = ps_in.rearrange("(t p) f -> t p f", p=P)
    m_t = m_out.rearrange("(t p) f -> t p f", p=P)
    nps_t = nps_out.rearrange("(t p) f -> t p f", p=P)

    with tile.TileContext(nc) as tc:
        with tc.tile_pool(name="res", bufs=1) as res:
          if True:
            pro = tc.alloc_tile_pool(name="pro", bufs=1)

            # ---------------- constants ----------------
            ones_colb = pro.tile([P, 1], bf16)
            nc.vector.memset(ones_colb, 1.0)
            ones_rowb = pro.tile([1, P], bf16)
            nc.vector.memset(ones_rowb, 1.0)
            # preload the ACT table set (Sqrt + fillers) off the critical path
            warm = pro.tile([1, 1], fp32)
            nc.vector.memset(warm, 1.0)
            nc.scalar.activation(warm, warm, Act.Sqrt)

            # PE p-state warm-up: a few throwaway matmuls so the Gram
            # starts at full clock instead of the cold 0.65GHz p-state
            wsc0 = pro.tile([P, FCW], bf16)
            nc.vector.memset(wsc0, 0.0)
            with tc.tile_pool(name="wup", bufs=1, space="PSUM") as wup:
                wp = wup.tile([P, FCW], fp32)
                for _ in range(8):
                    nc.tensor.matmul(wp, wsc0[:, 0:P], wsc0, start=True, stop=True)

            bnw_c = pro.tile([P, FP], fp32)
            nc.gpsimd.dma_start(bnw_c, bnw16)
            bnb_c = pro.tile([P, FP], fp32)
            nc.gpsimd.dma_start(bnb_c, bnb16)


            # ---------------- phase 1: FULL Gram, local (bf16) ----------
            # G_aug = a_aug^T a_aug over all N rows; column NA of a_aug is
            # ones, so column NA of G_aug is colsum(a).
            g0 = pro.tile([P, NA], bf16)
            g1 = pro.tile([P, NA], bf16)
            sc0 = pro.tile([P, 1], bf16)
            sc1 = pro.tile([P, 1], bf16)
            with tc.tile_pool(name="pro1", bufs=1, space="PSUM") as pp1, \
                 tc.tile_pool(name="abig", bufs=4) as abigp:
                DR = mybir.MatmulPerfMode.DoubleRow
                pg0 = pp1.tile([P, NAUGP], fp32)
                pg1 = pp1.tile([P, NAUGP], fp32)
                for g in range(GRP):
                    hch = abigp.tile([P, GJ2, 2, NAUGP], fp8, name="hch")
                    nc.sync.dma_start(hch, af_blk[g])
                    for j in range(GJ2):
                        first = g == 0 and j == 0
                        last = g == GRP - 1 and j == GJ2 - 1
                        a_t = hch[:, j, :, :]
                        nc.tensor.matmul(pg0, a_t[:, :, ts(0, P)], a_t,
                                         start=first, stop=last, perf_mode=DR)
                        nc.tensor.matmul(pg1, a_t[:, :, ts(1, P)], a_t,
                                         start=first, stop=last, perf_mode=DR)
                # W^T resident halves (phase 2 rhs + scale-fold source);
                # emitted after the af stream so they don't delay the Gram
                w0 = res.tile([P, F], bf16)
                nc.sync.dma_start(w0, wT[0:P, :])
                w1 = res.tile([P, F], bf16)
                nc.sync.dma_start(w1, wT[P:NA, :])
                # evict G + colsum as bf16
                for pg, gh, sch in ((pg0, g0, sc0), (pg1, g1, sc1)):
                    nc.vector.tensor_copy(gh, pg[:, 0:NA])
                    nc.scalar.copy(sch, pg[:, NA:NAUG])


            # ---------------- resident load of main-matmul lhsT ------------
            ah0 = res.tile([P, NSH], bf16)
            nc.sync.dma_start(ah0, ahT[0:P, :])
            ah1 = res.tile([P, NSH], bf16)
            nc.sync.dma_start(ah1, ahT[P:NA, :])

            # ---------------- phase 2: S1/S2 via H = G W^T ----------------
            st1r = pro.tile([1, F], fp32)     # S1 as a row (partition 0)
            st2r = pro.tile([1, F], fp32)     # S2 as a row (partition 0)
            with tc.tile_pool(name="pro2", bufs=1, space="PSUM") as pp2, \
                 tc.tile_pool(name="qtmp", bufs=2) as qtmp:
                for fc in range(FC):
                    fsl = ts(fc, FCW)
                    ph0 = pp2.tile([P, FCW], fp32, name="ph0", tag="ph0", bufs=2)
                    nc.tensor.matmul(ph0, g0[:, 0:P], w0[:, fsl], start=True, stop=False)
                    nc.tensor.matmul(ph0, g1[:, 0:P], w1[:, fsl], start=False, stop=True)
                    ph1 = pp2.tile([P, FCW], fp32, name="ph1", tag="ph1", bufs=2)
                    nc.tensor.matmul(ph1, g0[:, P:NA], w0[:, fsl], start=True, stop=False)
                    nc.tensor.matmul(ph1, g1[:, P:NA], w1[:, fsl], start=False, stop=True)
                    qf0 = qtmp.tile([P, FCW], fp32, name="qf0")
                    nc.vector.tensor_tensor(qf0, ph0, w0[:, fsl], Alu.mult)
                    qf1 = qtmp.tile([P, FCW], fp32, name="qf1")
                    nc.vector.tensor_tensor(qf1, ph1, w1[:, fsl], Alu.mult)
                    q0 = qtmp.tile([P, FCW], bf16, name="q0")
                    nc.scalar.copy(q0, qf0)
                    q1 = qtmp.tile([P, FCW], bf16, name="q1")
                    nc.scalar.copy(q1, qf1)
                    ps2 = pp2.tile([1, FCW], fp32, name="ps2", tag="ps2", bufs=2)
                    nc.tensor.matmul(ps2, ones_colb, q0, start=True, stop=False)
                    nc.tensor.matmul(ps2, ones_colb, q1, start=False, stop=True)
                    ps1 = pp2.tile([1, FCW], fp32, name="ps1", tag="ps1", bufs=2)
                    nc.tensor.matmul(ps1, sc0, w0[:, fsl], start=True, stop=False)
                    nc.tensor.matmul(ps1, sc1, w1[:, fsl], start=False, stop=True)
                    nc.scalar.copy(st1r[0:1, fsl], ps1)
                    nc.scalar.copy(st2r[0:1, fsl], ps2)

            # Redistribute the [1, F] rows into the [128, 16] stats layout
            # via SBUF->SBUF DMA (partition-scatter)
            st1c = pro.tile([P, FP], fp32)
            nc.gpsimd.dma_start(st1c, st1r)
            st2c = pro.tile([P, FP], fp32)
            nc.gpsimd.dma_start(st2c, st2r)

            # ---------------- phase 4: stats math in [128,16] layout --------
            sh_row = pro.tile([1, F], bf16)
            ttl = res.tile([1, F], bf16)        # folded bias row t
            ones1 = res.tile([1, P], bf16)
            nc.vector.memset(ones1, 1.0)
            with tc.tile_pool(name="smath", bufs=1) as sm:
                sq = sm.tile([P, FP], fp32)
                nc.vector.tensor_tensor(sq, st1c, st1c, Alu.mult)
                # vv = S2 - S1^2/N + N*eps  (= N*(var+eps))
                vv = sm.tile([P, FP], fp32)
                nc.vector.scalar_tensor_tensor(vv, sq, -1.0 / N, st2c, Alu.mult, Alu.add)
                nc.vector.tensor_scalar_add(vv, vv, float(N * BN_EPS))
                rr = sm.tile([P, FP], fp32)
                nc.scalar.activation(rr, vv, Act.Sqrt)
                y = sm.tile([P, FP], fp32)
                nc.vector.reciprocal(y, rr)
                # Newton iteration for 1/sqrt(vv) (ScalarE Sqrt is low-precision)
                for _ in range(1):
                    yy = sm.tile([P, FP], fp32, name="yy", tag="yy", bufs=2)
                    nc.vector.tensor_tensor(yy, y, y, Alu.mult)
                    vyy = sm.tile([P, FP], fp32, name="vyy", tag="vyy", bufs=2)
                    nc.vector.tensor_tensor(vyy, vv, yy, Alu.mult)
                    w = sm.tile([P, FP], fp32, name="w", tag="w", bufs=2)
                    nc.vector.tensor_scalar(w, vyy, -0.5, 1.5, Alu.mult, Alu.add)
                    y2 = sm.tile([P, FP], fp32, name="y2", tag="y2", bufs=2)
                    nc.vector.tensor_tensor(y2, y, w, Alu.mult)
                    y = y2
                # s = sqrt(N) * y * bn_w; folded bias t = bn_b - (S1/N)*s
                s_c = sm.tile([P, FP], fp32)
                nc.vector.scalar_tensor_tensor(s_c, y, float(np.sqrt(N)), bnw_c, Alu.mult, Alu.mult)
                tm = sm.tile([P, FP], fp32)
                nc.vector.scalar_tensor_tensor(tm, st1c, -1.0 / N, s_c, Alu.mult, Alu.mult)
                t_c = sm.tile([P, FP], fp32)
                nc.vector.tensor_tensor(t_c, tm, bnb_c, Alu.add)
                sh_c = sm.tile([P, FP], bf16)
                nc.vector.tensor_copy(sh_c, s_c)
                th_c = sm.tile([P, FP], bf16)
                nc.vector.tensor_copy(th_c, t_c)
                nc.gpsimd.dma_start(sh_row, sh_c)
                nc.gpsimd.dma_start(ttl, th_c)

            # ---------------- phase 5: fold scale into W^T ----------------
            w0h = res.tile([P, F], bf16)
            w1h = res.tile([P, F], bf16)
            with tc.tile_pool(name="pro3", bufs=2, space="PSUM") as pp3, \
                 tc.tile_pool(name="wsc", bufs=2) as wsc:
                for fc in range(FC):
                    fsl = ts(fc, FCW)
                    pb = pp3.tile([P, FCW], fp32, name="pb")
                    nc.tensor.matmul(pb, ones_rowb, sh_row[:, fsl], start=True, stop=True)
                    nc.vector.tensor_tensor(w0h[:, fsl], w0[:, fsl], pb, Alu.mult)
                    nc.vector.tensor_tensor(w1h[:, fsl], w1[:, fsl], pb, Alu.mult)

            pro.release()

            # ---------------- main loop over 16 row-tiles ----------------
            with tc.tile_pool(name="mx", bufs=8, space="PSUM") as mxp, \
                 tc.tile_pool(name="psb", bufs=8) as psb, \
                 tc.tile_pool(name="zb", bufs=4) as zb, \
                 tc.tile_pool(name="mb", bufs=3) as mb, \
                 tc.tile_pool(name="qb", bufs=3) as qb, \
                 tc.tile_pool(name="nb", bufs=3) as nb, \
                 tc.tile_pool(name="rsb", bufs=4) as rsb:
                for rt in range(RT):
                    rsl = ts(rt, P)
                    pst = psb.tile([P, F], bf16, name="pst")
                    nc.sync.dma_start(pst, ps_t[rt])
                    # fp16 zt: 2^-11 rounding is below the bf16 noise floor but
                    # unlocks DVE 2x for the downstream all-16-bit ops
                    zt = zb.tile([P, F], fp16, name="zt")
                    # pass-type-major: each lhsT is loaded once per row-tile and
                    # streams all 4 feature chunks (LDWEIGHTS dedupe-friendly)
                    px = mxp.tile([P, F], fp32, name="px", tag="px", bufs=2)
                    ptypes = [(ah0[:, rsl], w0h), (ah1[:, rsl], w1h),
                              (ones1, ttl)]
                    for pi, (lhsT, rhs) in enumerate(ptypes):
                        for fc in range(FC):
                            nc.tensor.matmul(px[:, ts(fc, FCW)], lhsT, rhs[:, ts(fc, FCW)],
                                             start=(pi == 0), stop=(pi == len(ptypes) - 1))
                    # z' = -xn * ps over the whole row-tile; rs = rowsum(z')
                    rs = rsb.tile([P, 1], fp32, name="rs")
                    nc.vector.scalar_tensor_tensor(
                        zt, px, -1.0, pst, Alu.mult, Alu.mult, accum_out=rs,
                    )
                    # rs = -sum(z); tau = (sum(z)+1)/2047 = (1-rs)/2047
                    ntau = rsb.tile([P, 1], fp32, name="ntau")      # -tau
                    nc.vector.tensor_scalar(ntau, rs, INV_D1, -INV_D1, Alu.mult, Alu.add)
                    # m = relu(z - tau) = relu(-z' + ntau); fp16 out (and
                    # fp16 m_out) so ut below runs in the DVE 4x mode
                    mt = mb.tile([P, F], fp16, name="mt")
                    nc.scalar.activation(mt, zt, Act.Relu, bias=ntau, scale=-1.0)
                    nc.scalar.dma_start(m_t[rt], mt)
                    # GAMMA - m  (exact: m = relu(z-tau) implies m >= 0)
                    ut = qb.tile([P, F], bf16, name="ut")
                    nc.vector.tensor_scalar(ut, mt, -1.0, GAMMA, Alu.mult, Alu.add)
                    nt = nb.tile([P, F], bf16, name="nt")
                    if rt % 4 == 1:
                        # Pool's tensor_tensor-mult is its one tuned op (~4us);
                        # more than 4 tiles on Pool backs up its queue
                        nc.gpsimd.tensor_tensor(nt, ut, pst, Alu.mult)
                    else:
                        # all-bf16/fp16 operands: DVE 2x mode (~1.2us)
                        nc.vector.tensor_tensor(nt, ut, pst, Alu.mult)
                    nc.scalar.dma_start(nps_t[rt], nt)

    nc.compile()
    return nc


def _get_nc():
    if "nc" not in _CACHE:
        _CACHE["nc"] = _build_bass()
    return _CACHE["nc"]


def _make_in_maps(a, ps, W, b, bn_w, bn_b):
    import ml_dtypes
    bf = ml_dtypes.bfloat16
    a = np.ascontiguousarray(a, dtype=np.float32)
    ah = a.astype(bf)
    f8 = ml_dtypes.float8_e4m3
    NAUGP = 272
    af_aug = np.concatenate(
        [a.astype(f8), np.ones((N, 1), f8),
         np.zeros((N, NAUGP - NAUG), f8)], axis=1)
    # blocked fp8 layout for DoubleRow: row = (g*GJ2 + j)*256 + sub*128 + p
    GRP = 16
    GJ2 = N // (GRP * 256)
    af_blk = np.ascontiguousarray(
        af_aug.reshape(GRP, GJ2, 2, P, NAUGP).transpose(0, 3, 1, 2, 4))
    wT_np = np.ascontiguousarray(W.astype(np.float32).T.astype(bf))
    ps16 = np.ascontiguousarray(ps, dtype=np.float32).astype(bf)
    bnw16 = np.ascontiguousarray(bn_w.astype(np.float32).reshape(P, FP))
    bnb16 = np.ascontiguousarray(bn_b.astype(np.float32).reshape(P, FP))
    in_maps = []
    for c in range(NCORES):
        rows = slice(c * NSH, (c + 1) * NSH)
        in_maps.append({
            "af_blk": af_blk,
            "ahT": np.ascontiguousarray(ah[rows].T),
            "wT": wT_np,
            "ps_in": np.ascontiguousarray(ps16[rows]),
            "bnw16": bnw16,
            "bnb16": bnb16,
        })
    return in_maps


def run(a, ps, W, b, bn_w, bn_b, trace=False, **kw):
    """Run the kernel on the 8 NeuronCores; returns ((m, new_ps), BassKernelResults)."""
    from concourse import bass_utils

    nc = _get_nc()
    in_maps = _make_in_maps(a, ps, W, b, bn_w, bn_b)
    res = bass_utils.run_bass_kernel_spmd(
        nc, in_maps, core_ids=list(range(NCORES)), trace=trace, **kw,
    )
    m = np.concatenate([np.asarray(r["m_out"]) for r in res.results],
                       axis=0).astype(np.float32)
    nps = np.concatenate([np.asarray(r["nps_out"]) for r in res.results],
                         axis=0).astype(np.float32)
    return (m, nps), res


def kernel(a, ps, W, b, bn_w, bn_b):
    (m, nps), _ = run(a, ps, W, b, bn_w, bn_b, trace=False)
    return m, nps


if __name__ == "__main__":
    rng = np.random.default_rng(0)
    a = rng.standard_normal((N, NA), dtype=np.float32)
    ps = rng.random((N, F), dtype=np.float32)
    lim = 1.0 / np.sqrt(NA)
    W = rng.uniform(-lim, lim, (F, NA)).astype(np.float32)
    b = rng.uniform(-lim, lim, (F,)).astype(np.float32)
    bn_w = np.ones((F, ), np.float32)
    bn_b = np.zeros((F, ), np.float32)
    (m, nps), res = run(a, ps, W, b, bn_w, bn_b)
    print("m", m.shape, m.dtype, "nps", nps.shape)
    print("exec_time_ns:", res.exec_time_ns)


# revision 30
# speedup vs baseline: 1.3359x; 1.0239x over previous
"""Trainium2 Bass kernel for nn_AttentiveTransformer (TabNet attentive transformer).

Computes, for full inputs (N=16384, NA=256, F=2048):
    x  = a @ W.T + b
    xn = batchnorm(x)  (training mode, batch stats over all N rows)
    m  = sparsemax_ascending_variant(xn * ps)
    new_ps = ps * (1.5 - m)

Key identities:
 * The reference "sparsemax" sorts ascending; its k_z condition always fires
   at the top index, so k_z = D-1 exactly and tau = (sum(z)+1)/(D-1),
   m = relu(z - tau). No sort needed.
 * BN stats from Gram partials: S1[f] = colsum(a).W_f, S2[f] = diag(W G W^T).
   var = S2/N - (S1/N)^2; the affine normalization folds into the matmul:
   W' = W*s, bias t = bn_b - (S1/N)*s (b cancels).
 * Mixed low precision, validated by exact CPU emulation (rel err 7.0e-3 /
   8.7e-3 vs the 2e-2 gate): bf16 operands everywhere, fp8+DoubleRow for
   the Gram (one matmul contracts 256 rows at 0.5 cyc/col), fp16 z/m tiles
   (2-byte for DVE fast modes, 4x less rounding than bf16), fp16 m / bf16
   nps outputs, fp32 PSUM + stats.
 * No collective: every core computes the FULL Gram locally from the whole
   a matrix (4.5MB in fp8, blocked so DMA descriptors stay 2KB+). The 16KB
   AllReduce this replaces costs ~97us of fixed barrier+trigger+mesh
   latency on this platform (measured with a bare-AllReduce microbench).
 * Engine budget per row-tile: PE 8 data + 4 bias matmuls (~4.5us at the
   1.37GHz the HW actually sustains), DVE z'/ut/most nt (~3.6us), ACT
   relu + store issues (~3.2us), Pool 4 of 16 nt (more backs up its
   queue). Loads and stores are issued on different DMA queues (queues
   drain in order; a store waiting on compute must never block prefetch).

Sharding: data-parallel over rows for the main pass, 2048 rows/core.
Timeline: 283.7us baseline -> 150us (preamble 6 + fp8 Gram 21 + stats 14
+ PE-paced main loop ~65 + drain tail).
"""

import os
import sys
import numpy as np

for _p in ("/opt/trn_rl_repo",):
    if _p not in sys.path:
        sys.path.insert(0, _p)

KVAR = os.environ.get("KVAR", "")             # debug variant flags

N, NA, F = 16384, 256, 2048
NCORES = 8
NSH = N // NCORES            # 2048 rows per core
P = 128                      # partitions
RT = NSH // P                # 16 row-tiles per core
FCW = 512                    # feature chunk width (psum bank / max moving free)
FC = F // FCW                # 4 feature chunks
FP = F // P                  # 16 (cols of the [128,16] stats layout)
NAUG = NA + 1                # 257: a with ones column (colsum rides the Gram)
GAMMA = 1.5
BN_EPS = 1e-5
INV_D1 = 1.0 / (F - 1.0)     # 1/2047
ACH = N // P                 # 128 row-chunks of the full a matrix

_CACHE = {}


def _build_bass():
    import concourse.mybir as mybir
    import concourse.tile as tile
    from concourse import bacc
    from concourse.bass import ts

    fp32 = mybir.dt.float32
    bf16 = mybir.dt.bfloat16
    fp16 = mybir.dt.float16
    Alu = mybir.AluOpType
    Act = mybir.ActivationFunctionType

    nc = bacc.Bacc(
        "TRN2",
        target_bir_lowering=False,
        debug=False,
        enable_asserts=False,
        num_devices=NCORES,
    )

    # I/O (per core). af_blk = FULL a in bf16 with a ones column appended
    # (replicated to every core; feeds the local full-Gram), pre-blocked on
    # the host as [group, partition, rows-per-group, col] so each DMA lands
    # 4KB-contiguous per partition (row-major a would give 514B descriptors).
    # The Gram doesn't care which rows share a partition chunk, only that
    # every row is accumulated exactly once. ahT = this core's row-shard,
    # transposed (main matmul lhsT).
    # fp8 + DoubleRow: each matmul contracts a 256-row chunk (2 stacked
    # 128-row k-subtiles) at 0.5 cyc/col. row = (g*GJ2 + j)*256 + sub*128 + p
    GRP, GJ2 = 16, N // (16 * 256)   # 16 groups x 4 chunks-of-256
    NAUGP = 272                      # 257 padded to 16-elem alignment for DR
    fp8 = mybir.dt.float8e4
    af_blk = nc.dram_tensor("af_blk", [GRP, P, GJ2, 2, NAUGP], fp8,
                            kind="ExternalInput").ap()
    ahT = nc.dram_tensor("ahT", [NA, NSH], bf16, kind="ExternalInput").ap()
    wT = nc.dram_tensor("wT", [NA, F], bf16, kind="ExternalInput").ap()
    ps_in = nc.dram_tensor("ps_in", [NSH, F], bf16, kind="ExternalInput").ap()
    bnw16 = nc.dram_tensor("bnw16", [P, FP], fp32, kind="ExternalInput").ap()
    bnb16 = nc.dram_tensor("bnb16", [P, FP], fp32, kind="ExternalInput").ap()
    m_out = nc.dram_tensor("m_out", [NSH, F], fp16, kind="ExternalOutput").ap()
    nps_out = nc.dram_tensor("nps_out", [NSH, F], bf16, kind="ExternalOutput").ap()

    ps_t = ps_in.rearrange("(t p) f -> t p f", p=P)
    m_t = m_out.rearrange("(t p) f -> t p f", p=P)
    nps_t = nps_out.rearrange("(t p) f -> t p f", p=P)

    with tile.TileContext(nc) as tc:
        with tc.tile_pool(name="res", bufs=1) as res:
          if True:
            pro = tc.alloc_tile_pool(name="pro", bufs=1)

            # ---------------- constants ----------------
            ones_colb = pro.tile([P, 1], bf16)
            nc.vector.memset(ones_colb, 1.0)
            ones_rowb = pro.tile([1, P], bf16)
            nc.vector.memset(ones_rowb, 1.0)
            # preload the ACT table set (Sqrt + fillers) off the critical path
            warm = pro.tile([1, 1], fp32)
            nc.vector.memset(warm, 1.0)
            nc.scalar.activation(warm, warm, Act.Sqrt)

            # PE p-state warm-up: a few throwaway matmuls so the Gram
            # starts at full clock instead of the cold 0.65GHz p-state
            wsc0 = pro.tile([P, FCW], bf16)
            nc.vector.memset(wsc0, 0.0)
            with tc.tile_pool(name="wup", bufs=1, space="PSUM") as wup:
                wp = wup.tile([P, FCW], fp32)
                for _ in range(8):
                    nc.tensor.matmul(wp, wsc0[:, 0:P], wsc0, start=True, stop=True)

            bnw_c = pro.tile([P, FP], fp32)
            nc.gpsimd.dma_start(bnw_c, bnw16)
            bnb_c = pro.tile([P, FP], fp32)
            nc.gpsimd.dma_start(bnb_c, bnb16)


            # ---------------- phase 1: FULL Gram, local (bf16) ----------
            # G_aug = a_aug^T a_aug over all N rows; column NA of a_aug is
            # ones, so column NA of G_aug is colsum(a).
            g0 = pro.tile([P, NA], bf16)
            g1 = pro.tile([P, NA], bf16)
            sc0 = pro.tile([P, 1], bf16)
            sc1 = pro.tile([P, 1], bf16)
            with tc.tile_pool(name="pro1", bufs=1, space="PSUM") as pp1, \
                 tc.tile_pool(name="abig", bufs=4) as abigp:
                DR = mybir.MatmulPerfMode.DoubleRow
                pg0 = pp1.tile([P, NAUGP], fp32)
                pg1 = pp1.tile([P, NAUGP], fp32)
                for g in range(GRP):
                    hch = abigp.tile([P, GJ2, 2, NAUGP], fp8, name="hch")
                    nc.sync.dma_start(hch, af_blk[g])
                    for j in range(GJ2):
                        first = g == 0 and j == 0
                        last = g == GRP - 1 and j == GJ2 - 1
                        a_t = hch[:, j, :, :]
                        nc.tensor.matmul(pg0, a_t[:, :, ts(0, P)], a_t,
                                         start=first, stop=last, perf_mode=DR)
                        nc.tensor.matmul(pg1, a_t[:, :, ts(1, P)], a_t,
                                         start=first, stop=last, perf_mode=DR)
                # W^T resident halves (phase 2 rhs + scale-fold source);
                # emitted after the af stream so they don't delay the Gram
                w0 = res.tile([P, F], bf16)
                nc.sync.dma_start(w0, wT[0:P, :])
                w1 = res.tile([P, F], bf16)
                nc.sync.dma_start(w1, wT[P:NA, :])
                # evict G + colsum as bf16
                for pg, gh, sch in ((pg0, g0, sc0), (pg1, g1, sc1)):
                    nc.vector.tensor_copy(gh, pg[:, 0:NA])
                    nc.scalar.copy(sch, pg[:, NA:NAUG])


            # ---------------- resident load of main-matmul lhsT ------------
            ah0 = res.tile([P, NSH], bf16)
            nc.sync.dma_start(ah0, ahT[0:P, :])
            ah1 = res.tile([P, NSH], bf16)
            nc.sync.dma_start(ah1, ahT[P:NA, :])

            # ---------------- phase 2: S1/S2 via H = G W^T ----------------
            st1r = pro.tile([1, F], fp32)     # S1 as a row (partition 0)
            st2r = pro.tile([1, F], fp32)     # S2 as a row (partition 0)
            with tc.tile_pool(name="pro2", bufs=1, space="PSUM") as pp2, \
                 tc.tile_pool(name="qtmp", bufs=2) as qtmp:
                for fc in range(FC):
                    fsl = ts(fc, FCW)
                    ph0 = pp2.tile([P, FCW], fp32, name="ph0", tag="ph0", bufs=2)
                    nc.tensor.matmul(ph0, g0[:, 0:P], w0[:, fsl], start=True, stop=False)
                    nc.tensor.matmul(ph0, g1[:, 0:P], w1[:, fsl], start=False, stop=True)
                    ph1 = pp2.tile([P, FCW], fp32, name="ph1", tag="ph1", bufs=2)
                    nc.tensor.matmul(ph1, g0[:, P:NA], w0[:, fsl], start=True, stop=False)
                    nc.tensor.matmul(ph1, g1[:, P:NA], w1[:, fsl], start=False, stop=True)
                    qf0 = qtmp.tile([P, FCW], fp32, name="qf0")
                    nc.vector.tensor_tensor(qf0, ph0, w0[:, fsl], Alu.mult)
                    qf1 = qtmp.tile([P, FCW], fp32, name="qf1")
                    nc.vector.tensor_tensor(qf1, ph1, w1[:, fsl], Alu.mult)
                    q0 = qtmp.tile([P, FCW], bf16, name="q0")
                    nc.scalar.copy(q0, qf0)
                    q1 = qtmp.tile([P, FCW], bf16, name="q1")
                    nc.scalar.copy(q1, qf1)
                    ps2 = pp2.tile([1, FCW], fp32, name="ps2", tag="ps2", bufs=2)
                    nc.tensor.matmul(ps2, ones_colb, q0, start=True, stop=False)
                    nc.tensor.matmul(ps2, ones_colb, q1, start=False, stop=True)
                    ps1 = pp2.tile([1, FCW], fp32, name="ps1", tag="ps1", bufs=2)
                    nc.tensor.matmul(ps1, sc0, w0[:, fsl], start=True, stop=False)
                    nc.tensor.matmul(ps1, sc1, w1[:, fsl], start=False, stop=True)
                    nc.scalar.copy(st1r[0:1, fsl], ps1)
                    nc.scalar.copy(st2r[0:1, fsl], ps2)

            # Redistribute the [1, F] rows into the [128, 16] stats layout
            # via SBUF->SBUF DMA (partition-scatter)
            st1c = pro.tile([P, FP], fp32)
            nc.gpsimd.dma_start(st1c, st1r)
            st2c = pro.tile([P, FP], fp32)
            nc.gpsimd.dma_start(st2c, st2r)

            # ---------------- phase 4: stats math in [128,16] layout --------
            sh_row = pro.tile([1, F], bf16)
            ttl = res.tile([1, F], bf16)        # folded bias row t
            ones1 = res.tile([1, P], bf16)
            nc.vector.memset(ones1, 1.0)
            with tc.tile_pool(name="smath", bufs=1) as sm:
                sq = sm.tile([P, FP], fp32)
                nc.vector.tensor_tensor(sq, st1c, st1c, Alu.mult)
                # vv = S2 - S1^2/N + N*eps  (= N*(var+eps))
                vv = sm.tile([P, FP], fp32)
                nc.vector.scalar_tensor_tensor(vv, sq, -1.0 / N, st2c, Alu.mult, Alu.add)
                nc.vector.tensor_scalar_add(vv, vv, float(N * BN_EPS))
                rr = sm.tile([P, FP], fp32)
                nc.scalar.activation(rr, vv, Act.Sqrt)
                y = sm.tile([P, FP], fp32)
                nc.vector.reciprocal(y, rr)
                # Newton iteration for 1/sqrt(vv) (ScalarE Sqrt is low-precision)
                for _ in range(1):
                    yy = sm.tile([P, FP], fp32, name="yy", tag="yy", bufs=2)
                    nc.vector.tensor_tensor(yy, y, y, Alu.mult)
                    vyy = sm.tile([P, FP], fp32, name="vyy", tag="vyy", bufs=2)
                    nc.vector.tensor_tensor(vyy, vv, yy, Alu.mult)
                    w = sm.tile([P, FP], fp32, name="w", tag="w", bufs=2)
                    nc.vector.tensor_scalar(w, vyy, -0.5, 1.5, Alu.mult, Alu.add)
                    y2 = sm.tile([P, FP], fp32, name="y2", tag="y2", bufs=2)
                    nc.vector.tensor_tensor(y2, y, w, Alu.mult)
                    y = y2
                # s = sqrt(N) * y * bn_w; folded bias t = bn_b - (S1/N)*s
                s_c = sm.tile([P, FP], fp32)
                nc.vector.scalar_tensor_tensor(s_c, y, float(np.sqrt(N)), bnw_c, Alu.mult, Alu.mult)
                tm = sm.tile([P, FP], fp32)
                nc.vector.scalar_tensor_tensor(tm, st1c, -1.0 / N, s_c, Alu.mult, Alu.mult)
                t_c = sm.tile([P, FP], fp32)
                nc.vector.tensor_tensor(t_c, tm, bnb_c, Alu.add)
                sh_c = sm.tile([P, FP], bf16)
                nc.vector.tensor_copy(sh_c, s_c)
                th_c = sm.tile([P, FP], bf16)
                nc.vector.tensor_copy(th_c, t_c)
                nc.gpsimd.dma_start(sh_row, sh_c)
                nc.gpsimd.dma_start(ttl, th_c)

            # ---------------- phase 5: fold scale into W^T ----------------
            w0h = res.tile([P, F], bf16)
            w1h = res.tile([P, F], bf16)
            with tc.tile_pool(name="pro3", bufs=2, space="PSUM") as pp3, \
                 tc.tile_pool(name="wsc", bufs=2) as wsc:
                for fc in range(FC):
                    fsl = ts(fc, FCW)
                    pb = pp3.tile([P, FCW], fp32, name="pb")
                    nc.tensor.matmul(pb, ones_rowb, sh_row[:, fsl], start=True, stop=True)
                    nc.vector.tensor_tensor(w0h[:, fsl], w0[:, fsl], pb, Alu.mult)
                    nc.vector.tensor_tensor(w1h[:, fsl], w1[:, fsl], pb, Alu.mult)

            pro.release()

            # ---------------- main loop over 16 row-tiles ----------------
            with tc.tile_pool(name="mx", bufs=8, space="PSUM") as mxp, \
                 tc.tile_pool(name="psb", bufs=8) as psb, \
                 tc.tile_pool(name="zb", bufs=4) as zb, \
                 tc.tile_pool(name="mb", bufs=3) as mb, \
                 tc.tile_pool(name="qb", bufs=3) as qb, \
                 tc.tile_pool(name="nb", bufs=3) as nb, \
                 tc.tile_pool(name="rsb", bufs=4) as rsb:
                for rt in range(RT):
                    rsl = ts(rt, P)
                    pst = psb.tile([P, F], bf16, name="pst")
                    nc.sync.dma_start(pst, ps_t[rt])
                    # fp16 zt: 2^-11 rounding is below the bf16 noise floor but
                    # unlocks DVE 2x for the downstream all-16-bit ops
                    zt = zb.tile([P, F], fp16, name="zt")
                    # pass-type-major: each lhsT is loaded once per row-tile and
                    # streams all 4 feature chunks (LDWEIGHTS dedupe-friendly)
                    px = mxp.tile([P, F], fp32, name="px", tag="px", bufs=2)
                    ptypes = [(ah0[:, rsl], w0h), (ah1[:, rsl], w1h),
                              (ones1, ttl)]
                    for pi, (lhsT, rhs) in enumerate(ptypes):
                        for fc in range(FC):
                            nc.tensor.matmul(px[:, ts(fc, FCW)], lhsT, rhs[:, ts(fc, FCW)],
                                             start=(pi == 0), stop=(pi == len(ptypes) - 1))
                    # z' = -xn * ps over the whole row-tile; rs = rowsum(z')
                    rs = rsb.tile([P, 1], fp32, name="rs")
                    nc.vector.scalar_tensor_tensor(
                        zt, px, -1.0, pst, Alu.mult, Alu.mult, accum_out=rs,
                    )
                    # rs = -sum(z); tau = (sum(z)+1)/2047 = (1-rs)/2047
                    ntau = rsb.tile([P, 1], fp32, name="ntau")      # -tau
                    nc.vector.tensor_scalar(ntau, rs, INV_D1, -INV_D1, Alu.mult, Alu.add)
                    # m = relu(z - tau) = relu(-z' + ntau); fp16 out (and
                    # fp16 m_out) so ut below runs in the DVE 4x mode
                    mt = mb.tile([P, F], fp16, name="mt")
                    nc.scalar.activation(mt, zt, Act.Relu, bias=ntau, scale=-1.0)
                    nc.scalar.dma_start(m_t[rt], mt)
                    # GAMMA - m  (exact: m = relu(z-tau) implies m >= 0)
                    ut = qb.tile([P, F], bf16, name="ut")
                    nc.vector.tensor_scalar(ut, mt, -1.0, GAMMA, Alu.mult, Alu.add)
                    nt = nb.tile([P, F], bf16, name="nt")
                    if rt % 4 == 1:
                        # Pool's tensor_tensor-mult is its one tuned op (~4us);
                        # more than 4 tiles on Pool backs up its queue
                        nc.gpsimd.tensor_tensor(nt, ut, pst, Alu.mult)
                    else:
                        # all-bf16/fp16 operands: DVE 2x mode (~1.2us)
                        nc.vector.tensor_tensor(nt, ut, pst, Alu.mult)
                    nc.scalar.dma_start(nps_t[rt], nt)

    nc.compile()
    return nc


def _get_nc():
    if "nc" not in _CACHE:
        _CACHE["nc"] = _build_bass()
    return _CACHE["nc"]


def _make_in_maps(a, ps, W, b, bn_w, bn_b):
    import ml_dtypes
    bf = ml_dtypes.bfloat16
    a = np.ascontiguousarray(a, dtype=np.float32)
    ah = a.astype(bf)
    f8 = ml_dtypes.float8_e4m3
    NAUGP = 272
    af_aug = np.concatenate(
        [a.astype(f8), np.ones((N, 1), f8),
         np.zeros((N, NAUGP - NAUG), f8)], axis=1)
    # blocked fp8 layout for DoubleRow: row = (g*GJ2 + j)*256 + sub*128 + p
    GRP = 16
    GJ2 = N // (GRP * 256)
    af_blk = np.ascontiguousarray(
        af_aug.reshape(GRP, GJ2, 2, P, NAUGP).transpose(0, 3, 1, 2, 4))
    wT_np = np.ascontiguousarray(W.astype(np.float32).T.astype(bf))
    ps16 = np.ascontiguousarray(ps, dtype=np.float32).astype(bf)
    bnw16 = np.ascontiguousarray(bn_w.astype(np.float32).reshape(P, FP))
    bnb16 = np.ascontiguousarray(bn_b.astype(np.float32).reshape(P, FP))
    in_maps = []
    for c in range(NCORES):
        rows = slice(c * NSH, (c + 1) * NSH)
        in_maps.append({
            "af_blk": af_blk,
            "ahT": np.ascontiguousarray(ah[rows].T),
            "wT": wT_np,
            "ps_in": np.ascontiguousarray(ps16[rows]),
            "bnw16": bnw16,
            "bnb16": bnb16,
        })
    return in_maps


def run(a, ps, W, b, bn_w, bn_b, trace=False, **kw):
    """Run the kernel on the 8 NeuronCores; returns ((m, new_ps), BassKernelResults)."""
    from concourse import bass_utils

    nc = _get_nc()
    in_maps = _make_in_maps(a, ps, W, b, bn_w, bn_b)
    res = bass_utils.run_bass_kernel_spmd(
        nc, in_maps, core_ids=list(range(NCORES)), trace=trace, **kw,
    )
    m = np.concatenate([np.asarray(r["m_out"]) for r in res.results],
                       axis=0).astype(np.float32)
    nps = np.concatenate([np.asarray(r["nps_out"]) for r in res.results],
                         axis=0).astype(np.float32)
    return (m, nps), res


def kernel(a, ps, W, b, bn_w, bn_b):
    (m, nps), _ = run(a, ps, W, b, bn_w, bn_b, trace=False)
    return m, nps


if __name__ == "__main__":
    rng = np.random.default_rng(0)
    a = rng.standard_normal((N, NA), dtype=np.float32)
    ps = rng.random((N, F), dtype=np.float32)
    lim = 1.0 / np.sqrt(NA)
    W = rng.uniform(-lim, lim, (F, NA)).astype(np.float32)
    b = rng.uniform(-lim, lim, (F,)).astype(np.float32)
    bn_w = np.ones((F, ), np.float32)
    bn_b = np.zeros((F, ), np.float32)
    (m, nps), res = run(a, ps, W, b, bn_w, bn_b)
    print("m", m.shape, m.dtype, "nps", nps.shape)
    print("exec_time_ns:", res.exec_time_ns)


# revision 31
# speedup vs baseline: 1.4087x; 1.0545x over previous
"""Trainium2 Bass kernel for nn_AttentiveTransformer (TabNet attentive transformer).

Computes, for full inputs (N=16384, NA=256, F=2048):
    x  = a @ W.T + b
    xn = batchnorm(x)  (training mode, batch stats over all N rows)
    m  = sparsemax_ascending_variant(xn * ps)
    new_ps = ps * (1.5 - m)

Key identities:
 * The reference "sparsemax" sorts ascending; its k_z condition always fires
   at the top index, so k_z = D-1 exactly and tau = (sum(z)+1)/(D-1),
   m = relu(z - tau). No sort needed.
 * BN stats from Gram partials: S1[f] = colsum(a).W_f, S2[f] = diag(W G W^T).
   var = S2/N - (S1/N)^2; the affine normalization folds into the matmul:
   W' = W*s, bias t = bn_b - (S1/N)*s (b cancels).
 * Mixed low precision, validated by exact CPU emulation (rel err 7.0e-3 /
   8.7e-3 vs the 2e-2 gate): bf16 operands everywhere, fp8+DoubleRow for
   the Gram (one matmul contracts 256 rows at 0.5 cyc/col), fp16 z/m tiles
   (2-byte for DVE fast modes, 4x less rounding than bf16), fp16 m / bf16
   nps outputs, fp32 PSUM + stats.
 * No collective: every core computes the FULL Gram locally from the whole
   a matrix (4.5MB in fp8, blocked so DMA descriptors stay 2KB+). The 16KB
   AllReduce this replaces costs ~97us of fixed barrier+trigger+mesh
   latency on this platform (measured with a bare-AllReduce microbench).
 * Engine budget per row-tile: PE 8 data + 4 bias matmuls (~4.5us at the
   1.37GHz the HW actually sustains), DVE z'/ut/most nt (~3.6us), ACT
   relu + store issues (~3.2us), Pool 4 of 16 nt (more backs up its
   queue). Loads and stores are issued on different DMA queues (queues
   drain in order; a store waiting on compute must never block prefetch).

Sharding: data-parallel over rows for the main pass, 2048 rows/core.
Timeline: 283.7us baseline -> 150us (preamble 6 + fp8 Gram 21 + stats 14
+ PE-paced main loop ~65 + drain tail).
"""

import os
import sys
import numpy as np

for _p in ("/opt/trn_rl_repo",):
    if _p not in sys.path:
        sys.path.insert(0, _p)

KVAR = os.environ.get("KVAR", "")             # debug variant flags

N, NA, F = 16384, 256, 2048
NCORES = 8
NSH = N // NCORES            # 2048 rows per core
P = 128                      # partitions
RT = NSH // P                # 16 row-tiles per core
FCW = 512                    # feature chunk width (psum bank / max moving free)
FC = F // FCW                # 4 feature chunks
FP = F // P                  # 16 (cols of the [128,16] stats layout)
NAUG = NA + 1                # 257: a with ones column (colsum rides the Gram)
GAMMA = 1.5
BN_EPS = 1e-5
INV_D1 = 1.0 / (F - 1.0)     # 1/2047
ACH = N // P                 # 128 row-chunks of the full a matrix

_CACHE = {}


def _build_bass():
    import concourse.mybir as mybir
    import concourse.tile as tile
    from concourse import bacc
    from concourse.bass import ts

    fp32 = mybir.dt.float32
    bf16 = mybir.dt.bfloat16
    fp16 = mybir.dt.float16
    Alu = mybir.AluOpType
    Act = mybir.ActivationFunctionType

    nc = bacc.Bacc(
        "TRN2",
        target_bir_lowering=False,
        debug=False,
        enable_asserts=False,
        num_devices=NCORES,
    )

    # I/O (per core). af_blk = FULL a in bf16 with a ones column appended
    # (replicated to every core; feeds the local full-Gram), pre-blocked on
    # the host as [group, partition, rows-per-group, col] so each DMA lands
    # 4KB-contiguous per partition (row-major a would give 514B descriptors).
    # The Gram doesn't care which rows share a partition chunk, only that
    # every row is accumulated exactly once. ahT = this core's row-shard,
    # transposed (main matmul lhsT).
    # fp8 + DoubleRow: each matmul contracts a 256-row chunk (2 stacked
    # 128-row k-subtiles) at 0.5 cyc/col. row = (g*GJ2 + j)*256 + sub*128 + p
    GRP, GJ2 = 16, N // (16 * 256)   # 16 groups x 4 chunks-of-256
    NAUGP = 272                      # 257 padded to 16-elem alignment for DR
    fp8 = mybir.dt.float8e4
    af_blk = nc.dram_tensor("af_blk", [GRP, P, GJ2, 2, NAUGP], fp8,
                            kind="ExternalInput").ap()
    ahT = nc.dram_tensor("ahT", [NA, NSH], bf16, kind="ExternalInput").ap()
    wT = nc.dram_tensor("wT", [NA, F], bf16, kind="ExternalInput").ap()
    ps_in = nc.dram_tensor("ps_in", [NSH, F], bf16, kind="ExternalInput").ap()
    bnw16 = nc.dram_tensor("bnw16", [P, FP], fp32, kind="ExternalInput").ap()
    bnb16 = nc.dram_tensor("bnb16", [P, FP], fp32, kind="ExternalInput").ap()
    m_out = nc.dram_tensor("m_out", [NSH, F], fp16, kind="ExternalOutput").ap()
    nps_out = nc.dram_tensor("nps_out", [NSH, F], bf16, kind="ExternalOutput").ap()

    ps_t = ps_in.rearrange("(t p) f -> t p f", p=P)
    m_t = m_out.rearrange("(t p) f -> t p f", p=P)
    nps_t = nps_out.rearrange("(t p) f -> t p f", p=P)

    with tile.TileContext(nc) as tc:
        with tc.tile_pool(name="res", bufs=1) as res:
          if True:
            pro = tc.alloc_tile_pool(name="pro", bufs=1)

            # ---------------- constants ----------------
            ones_colb = pro.tile([P, 1], bf16)
            nc.vector.memset(ones_colb, 1.0)
            ones_rowb = pro.tile([1, P], bf16)
            nc.vector.memset(ones_rowb, 1.0)
            # preload the ACT table set (Sqrt + fillers) off the critical path
            warm = pro.tile([1, 1], fp32)
            nc.vector.memset(warm, 1.0)
            nc.scalar.activation(warm, warm, Act.Sqrt)

            # PE p-state warm-up: a few throwaway matmuls so the Gram
            # starts at full clock instead of the cold 0.65GHz p-state
            wsc0 = pro.tile([P, FCW], bf16)
            nc.vector.memset(wsc0, 0.0)
            with tc.tile_pool(name="wup", bufs=1, space="PSUM") as wup:
                wp = wup.tile([P, FCW], fp32)
                for _ in range(8):
                    nc.tensor.matmul(wp, wsc0[:, 0:P], wsc0, start=True, stop=True)

            bnw_c = pro.tile([P, FP], fp32)
            nc.gpsimd.dma_start(bnw_c, bnw16)
            bnb_c = pro.tile([P, FP], fp32)
            nc.gpsimd.dma_start(bnb_c, bnb16)


            # ---------------- phase 1: FULL Gram, local (bf16) ----------
            # G_aug = a_aug^T a_aug over all N rows; column NA of a_aug is
            # ones, so column NA of G_aug is colsum(a).
            g0 = pro.tile([P, NA], bf16)
            g1 = pro.tile([P, NA], bf16)
            sc0 = pro.tile([P, 1], bf16)
            sc1 = pro.tile([P, 1], bf16)
            with tc.tile_pool(name="pro1", bufs=1, space="PSUM") as pp1, \
                 tc.tile_pool(name="abig", bufs=4) as abigp:
                DR = mybir.MatmulPerfMode.DoubleRow
                pg0 = pp1.tile([P, NAUGP], fp32)
                pg1 = pp1.tile([P, NAUGP], fp32)
                for g in range(GRP):
                    hch = abigp.tile([P, GJ2, 2, NAUGP], fp8, name="hch")
                    nc.sync.dma_start(hch, af_blk[g])
                    for j in range(GJ2):
                        first = g == 0 and j == 0
                        last = g == GRP - 1 and j == GJ2 - 1
                        a_t = hch[:, j, :, :]
                        nc.tensor.matmul(pg0, a_t[:, :, ts(0, P)], a_t,
                                         start=first, stop=last, perf_mode=DR)
                        nc.tensor.matmul(pg1, a_t[:, :, ts(1, P)], a_t,
                                         start=first, stop=last, perf_mode=DR)
                # W^T resident halves (phase 2 rhs + scale-fold source);
                # emitted after the af stream so they don't delay the Gram
                w0 = res.tile([P, F], bf16)
                nc.sync.dma_start(w0, wT[0:P, :])
                w1 = res.tile([P, F], bf16)
                nc.sync.dma_start(w1, wT[P:NA, :])
                # evict G + colsum as bf16
                for pg, gh, sch in ((pg0, g0, sc0), (pg1, g1, sc1)):
                    nc.vector.tensor_copy(gh, pg[:, 0:NA])
                    nc.scalar.copy(sch, pg[:, NA:NAUG])


            # ---------------- resident load of main-matmul lhsT ------------
            ah0 = res.tile([P, NSH], bf16)
            nc.sync.dma_start(ah0, ahT[0:P, :])
            ah1 = res.tile([P, NSH], bf16)
            nc.sync.dma_start(ah1, ahT[P:NA, :])

            # ---------------- phase 2: S1/S2 via H = G W^T ----------------
            st1r = pro.tile([1, F], fp32)     # S1 as a row (partition 0)
            st2r = pro.tile([1, F], fp32)     # S2 as a row (partition 0)
            with tc.tile_pool(name="pro2", bufs=1, space="PSUM") as pp2, \
                 tc.tile_pool(name="qtmp", bufs=2) as qtmp:
                for fc in range(FC):
                    fsl = ts(fc, FCW)
                    ph0 = pp2.tile([P, FCW], fp32, name="ph0", tag="ph0", bufs=2)
                    nc.tensor.matmul(ph0, g0[:, 0:P], w0[:, fsl], start=True, stop=False)
                    nc.tensor.matmul(ph0, g1[:, 0:P], w1[:, fsl], start=False, stop=True)
                    ph1 = pp2.tile([P, FCW], fp32, name="ph1", tag="ph1", bufs=2)
                    nc.tensor.matmul(ph1, g0[:, P:NA], w0[:, fsl], start=True, stop=False)
                    nc.tensor.matmul(ph1, g1[:, P:NA], w1[:, fsl], start=False, stop=True)
                    qf0 = qtmp.tile([P, FCW], fp32, name="qf0")
                    nc.vector.tensor_tensor(qf0, ph0, w0[:, fsl], Alu.mult)
                    qf1 = qtmp.tile([P, FCW], fp32, name="qf1")
                    nc.vector.tensor_tensor(qf1, ph1, w1[:, fsl], Alu.mult)
                    q0 = qtmp.tile([P, FCW], bf16, name="q0")
                    nc.scalar.copy(q0, qf0)
                    q1 = qtmp.tile([P, FCW], bf16, name="q1")
                    nc.scalar.copy(q1, qf1)
                    ps2 = pp2.tile([1, FCW], fp32, name="ps2", tag="ps2", bufs=2)
                    nc.tensor.matmul(ps2, ones_colb, q0, start=True, stop=False)
                    nc.tensor.matmul(ps2, ones_colb, q1, start=False, stop=True)
                    ps1 = pp2.tile([1, FCW], fp32, name="ps1", tag="ps1", bufs=2)
                    nc.tensor.matmul(ps1, sc0, w0[:, fsl], start=True, stop=False)
                    nc.tensor.matmul(ps1, sc1, w1[:, fsl], start=False, stop=True)
                    nc.scalar.copy(st1r[0:1, fsl], ps1)
                    nc.scalar.copy(st2r[0:1, fsl], ps2)

            # Redistribute the [1, F] rows into the [128, 16] stats layout
            # via SBUF->SBUF DMA (partition-scatter)
            st1c = pro.tile([P, FP], fp32)
            nc.gpsimd.dma_start(st1c, st1r)
            st2c = pro.tile([P, FP], fp32)
            nc.gpsimd.dma_start(st2c, st2r)

            # ---------------- phase 4: stats math in [128,16] layout --------
            sh_row = pro.tile([1, F], bf16)
            ttl = res.tile([1, F], bf16)        # folded bias row t
            ones1 = res.tile([1, P], bf16)
            nc.vector.memset(ones1, 1.0)
            with tc.tile_pool(name="smath", bufs=1) as sm:
                sq = sm.tile([P, FP], fp32)
                nc.vector.tensor_tensor(sq, st1c, st1c, Alu.mult)
                # vv = S2 - S1^2/N + N*eps  (= N*(var+eps))
                vv = sm.tile([P, FP], fp32)
                nc.vector.scalar_tensor_tensor(vv, sq, -1.0 / N, st2c, Alu.mult, Alu.add)
                nc.vector.tensor_scalar_add(vv, vv, float(N * BN_EPS))
                rr = sm.tile([P, FP], fp32)
                nc.scalar.activation(rr, vv, Act.Sqrt)
                y = sm.tile([P, FP], fp32)
                nc.vector.reciprocal(y, rr)
                # Newton iteration for 1/sqrt(vv) (ScalarE Sqrt is low-precision)
                for _ in range(1):
                    yy = sm.tile([P, FP], fp32, name="yy", tag="yy", bufs=2)
                    nc.vector.tensor_tensor(yy, y, y, Alu.mult)
                    vyy = sm.tile([P, FP], fp32, name="vyy", tag="vyy", bufs=2)
                    nc.vector.tensor_tensor(vyy, vv, yy, Alu.mult)
                    w = sm.tile([P, FP], fp32, name="w", tag="w", bufs=2)
                    nc.vector.tensor_scalar(w, vyy, -0.5, 1.5, Alu.mult, Alu.add)
                    y2 = sm.tile([P, FP], fp32, name="y2", tag="y2", bufs=2)
                    nc.vector.tensor_tensor(y2, y, w, Alu.mult)
                    y = y2
                # s = sqrt(N) * y * bn_w; folded bias t = bn_b - (S1/N)*s
                s_c = sm.tile([P, FP], fp32)
                nc.vector.scalar_tensor_tensor(s_c, y, float(np.sqrt(N)), bnw_c, Alu.mult, Alu.mult)
                tm = sm.tile([P, FP], fp32)
                nc.vector.scalar_tensor_tensor(tm, st1c, -1.0 / N, s_c, Alu.mult, Alu.mult)
                t_c = sm.tile([P, FP], fp32)
                nc.vector.tensor_tensor(t_c, tm, bnb_c, Alu.add)
                sh_c = sm.tile([P, FP], bf16)
                nc.vector.tensor_copy(sh_c, s_c)
                th_c = sm.tile([P, FP], bf16)
                nc.vector.tensor_copy(th_c, t_c)
                nc.gpsimd.dma_start(sh_row, sh_c)
                nc.gpsimd.dma_start(ttl, th_c)

            # ---------------- phase 5: fold scale into W^T ----------------
            w0h = res.tile([P, F], bf16)
            w1h = res.tile([P, F], bf16)
            with tc.tile_pool(name="pro3", bufs=2, space="PSUM") as pp3, \
                 tc.tile_pool(name="wsc", bufs=2) as wsc:
                for fc in range(FC):
                    fsl = ts(fc, FCW)
                    pb = pp3.tile([P, FCW], fp32, name="pb")
                    nc.tensor.matmul(pb, ones_rowb, sh_row[:, fsl], start=True, stop=True)
                    nc.vector.tensor_tensor(w0h[:, fsl], w0[:, fsl], pb, Alu.mult)
                    nc.vector.tensor_tensor(w1h[:, fsl], w1[:, fsl], pb, Alu.mult)

            pro.release()

            # ---------------- main loop over 16 row-tiles ----------------
            with tc.tile_pool(name="mx", bufs=8, space="PSUM") as mxp, \
                 tc.tile_pool(name="psb", bufs=8) as psb, \
                 tc.tile_pool(name="zb", bufs=6) as zb, \
                 tc.tile_pool(name="mb", bufs=5) as mb, \
                 tc.tile_pool(name="qb", bufs=5) as qb, \
                 tc.tile_pool(name="nb", bufs=5) as nb, \
                 tc.tile_pool(name="rsb", bufs=8) as rsb:
                for rt in range(RT):
                    rsl = ts(rt, P)
                    pst = psb.tile([P, F], bf16, name="pst")
                    nc.sync.dma_start(pst, ps_t[rt])
                    # fp16 zt: 2^-11 rounding is below the bf16 noise floor but
                    # unlocks DVE 2x for the downstream all-16-bit ops
                    zt = zb.tile([P, F], fp16, name="zt")
                    # pass-type-major: each lhsT is loaded once per row-tile and
                    # streams all 4 feature chunks (LDWEIGHTS dedupe-friendly)
                    px = mxp.tile([P, F], fp32, name="px", tag="px", bufs=2)
                    ptypes = [(ah0[:, rsl], w0h), (ah1[:, rsl], w1h),
                              (ones1, ttl)]
                    for pi, (lhsT, rhs) in enumerate(ptypes):
                        for fc in range(FC):
                            nc.tensor.matmul(px[:, ts(fc, FCW)], lhsT, rhs[:, ts(fc, FCW)],
                                             start=(pi == 0), stop=(pi == len(ptypes) - 1))
                    # z' = -xn * ps over the whole row-tile; rs = rowsum(z')
                    rs = rsb.tile([P, 1], fp32, name="rs")
                    nc.vector.scalar_tensor_tensor(
                        zt, px, -1.0, pst, Alu.mult, Alu.mult, accum_out=rs,
                    )
                    # rs = -sum(z); tau = (sum(z)+1)/2047 = (1-rs)/2047
                    ntau = rsb.tile([P, 1], fp32, name="ntau")      # -tau
                    nc.vector.tensor_scalar(ntau, rs, INV_D1, -INV_D1, Alu.mult, Alu.add)
                    # m = relu(z - tau) = relu(-z' + ntau); fp16 out (and
                    # fp16 m_out) so ut below runs in the DVE 4x mode
                    mt = mb.tile([P, F], fp16, name="mt")
                    nc.scalar.activation(mt, zt, Act.Relu, bias=ntau, scale=-1.0)
                    nc.scalar.dma_start(m_t[rt], mt)
                    # GAMMA - m  (exact: m = relu(z-tau) implies m >= 0)
                    ut = qb.tile([P, F], bf16, name="ut")
                    nc.vector.tensor_scalar(ut, mt, -1.0, GAMMA, Alu.mult, Alu.add)
                    nt = nb.tile([P, F], bf16, name="nt")
                    if rt % 4 == 1:
                        # Pool's tensor_tensor-mult is its one tuned op (~4us);
                        # more than 4 tiles on Pool backs up its queue
                        nc.gpsimd.tensor_tensor(nt, ut, pst, Alu.mult)
                    else:
                        # all-bf16/fp16 operands: DVE 2x mode (~1.2us)
                        nc.vector.tensor_tensor(nt, ut, pst, Alu.mult)
                    # gpsimd queue: an ACT-queue store here would wait on
                    # nt and head-of-line-block the next tile's mt
                    nc.gpsimd.dma_start(nps_t[rt], nt)

    nc.compile()
    return nc


def _get_nc():
    if "nc" not in _CACHE:
        _CACHE["nc"] = _build_bass()
    return _CACHE["nc"]


def _make_in_maps(a, ps, W, b, bn_w, bn_b):
    import ml_dtypes
    bf = ml_dtypes.bfloat16
    a = np.ascontiguousarray(a, dtype=np.float32)
    ah = a.astype(bf)
    f8 = ml_dtypes.float8_e4m3
    NAUGP = 272
    af_aug = np.concatenate(
        [a.astype(f8), np.ones((N, 1), f8),
         np.zeros((N, NAUGP - NAUG), f8)], axis=1)
    # blocked fp8 layout for DoubleRow: row = (g*GJ2 + j)*256 + sub*128 + p
    GRP = 16
    GJ2 = N // (GRP * 256)
    af_blk = np.ascontiguousarray(
        af_aug.reshape(GRP, GJ2, 2, P, NAUGP).transpose(0, 3, 1, 2, 4))
    wT_np = np.ascontiguousarray(W.astype(np.float32).T.astype(bf))
    ps16 = np.ascontiguousarray(ps, dtype=np.float32).astype(bf)
    bnw16 = np.ascontiguousarray(bn_w.astype(np.float32).reshape(P, FP))
    bnb16 = np.ascontiguousarray(bn_b.astype(np.float32).reshape(P, FP))
    in_maps = []
    for c in range(NCORES):
        rows = slice(c * NSH, (c + 1) * NSH)
        in_maps.append({
            "af_blk": af_blk,
            "ahT": np.ascontiguousarray(ah[rows].T),
            "wT": wT_np,
            "ps_in": np.ascontiguousarray(ps16[rows]),
            "bnw16": bnw16,
            "bnb16": bnb16,
        })
    return in_maps


def run(a, ps, W, b, bn_w, bn_b, trace=False, **kw):
    """Run the kernel on the 8 NeuronCores; returns ((m, new_ps), BassKernelResults)."""
    from concourse import bass_utils

    nc = _get_nc()
    in_maps = _make_in_maps(a, ps, W, b, bn_w, bn_b)
    res = bass_utils.run_bass_kernel_spmd(
        nc, in_maps, core_ids=list(range(NCORES)), trace=trace, **kw,
    )
    m = np.concatenate([np.asarray(r["m_out"]) for r in res.results],
                       axis=0).astype(np.float32)
    nps = np.concatenate([np.asarray(r["nps_out"]) for r in res.results],
                         axis=0).astype(np.float32)
    return (m, nps), res


def kernel(a, ps, W, b, bn_w, bn_b):
    (m, nps), _ = run(a, ps, W, b, bn_w, bn_b, trace=False)
    return m, nps


if __name__ == "__main__":
    rng = np.random.default_rng(0)
    a = rng.standard_normal((N, NA), dtype=np.float32)
    ps = rng.random((N, F), dtype=np.float32)
    lim = 1.0 / np.sqrt(NA)
    W = rng.uniform(-lim, lim, (F, NA)).astype(np.float32)
    b = rng.uniform(-lim, lim, (F,)).astype(np.float32)
    bn_w = np.ones((F, ), np.float32)
    bn_b = np.zeros((F, ), np.float32)
    (m, nps), res = run(a, ps, W, b, bn_w, bn_b)
    print("m", m.shape, m.dtype, "nps", nps.shape)
    print("exec_time_ns:", res.exec_time_ns)
